# revision 27
# baseline (speedup 1.0000x reference)
"""MetaBaseline (retrieval_knn) Trainium2 kernel — bf16 edition.

Computation (per episode b):
  q  = l2norm(input1[b])            # [75, 25, 640] over channel
  s  = l2norm(input2[b])            # [5, 5, 25, 640]
  att = softmax_hw(s @ rpn_w)       # rpn_b is softmax-invariant
  cg  = leaky(sum_hw(att * s))
  feat = mean_shot(mean_hw(s) + 5 * cg)
  sim[b] = mean_hw(q) @ feat.T      # [75, 5]

Sharding: data-parallel over episodes, 4 per core on 8 cores.

Design notes (v2):
- All bulk data bf16 (host-rounded): halves HBM traffic (the kernel is
  memory-regime), enables the DVE 2x packed mode for the square-accum
  passes, and PE streams bf16 rhs at 1 col/cycle like f32r.
- One FIFO SWDGE ring (gpsimd) carries every big load in arrival order
  s0,q0a,q0b,s1,q1a,... so episode e's support data lands before its
  query chunks and episodes complete in order at full aggregate DMA
  bandwidth. Later issues are interleaved with Pool compute to keep the
  ring backlog bounded. Everything is SBUF-resident (~120KB/partition).
- Square-accum norm passes are split across ACT (Square+accum), DVE
  (scalar_tensor_tensor 2x), and Pool; support logits run on Pool.
- rsqrt = Exp(-0.5*Ln(x)) on ACT (2 ops per batch; table set
  natural_log_exp_and_others holds Ln/Exp/Square/Copy/Prelu). The 1/25
  hw-mean factor rides in the Exp bias for the query side.
- Support att-sum and hw-mean fuse into ONE matmul per slot-half with a
  concatenated [ut|mt] mask -> [50, 320] psum; evacuation fuses
  psum-read * softmax-recip * leaky into a single ACT Prelu per half.
- Staircase masks / identity / shot-mean matrices are host-precomputed
  and DMA'd (tiny) instead of burning gpsimd affine_selects.
"""

import sys
from contextlib import ExitStack

sys.path.insert(0, "/opt/trn_rl_repo")

import ml_dtypes
import numpy as np

import concourse.bass as bass
import concourse.tile as tile
from concourse import bacc, mybir
from concourse.bass_utils import run_bass_kernel_spmd

F32 = mybir.dt.float32
BF16 = mybir.dt.bfloat16
F8 = mybir.dt.float8e4      # scratch-only: halves useless SBUF writes
OP = mybir.AluOpType
AF = mybir.ActivationFunctionType

# Problem constants (fixed by the problem statement).
B, QN, WAY, SHOT, HH, WW, C = 32, 75, 5, 5, 5, 5, 640
NCORES = 8
E = B // NCORES        # 4 episodes per core
HW = HH * WW           # 25 spatial positions
P = 125                # descriptors per tile (5 groups of 25)
QT = 15                # query slots / episode (1875 = 15*125)
ST = 5                 # support slots / episode (625 = 5*125)
NMAP = WAY * SHOT      # 25 support maps / episode
GAMMA = 5.0
SLOPE = 0.01
CH = C // 2            # 320-column psum-bank-sized chunk
QA = 8                 # query slots in DMA chunk a (slots 0..7)
QB = QT - QA           # chunk b (slots 8..14)

# masks tensor column layout: 15 qmasks | 5 smasks | ident75 | shotm
MQ0 = 0                       # qmask_j at MQ0 + j*QN, width QN
MS0 = QT * QN                 # smask_j at MS0 + j*NMAP, width NMAP
MI0 = MS0 + ST * NMAP         # identity 75x75
MF0 = MI0 + QN                # shotm [25, 5]
MCOLS = MF0 + WAY

# norm-pass engine schedule (which engine squares+accumulates each slot).
# Pool cannot reduce over the free axis, so only ACT/DVE carry these.
# Measured: ACT Square+acc-read ~0.91us, DVE STT ~0.74us per [125,640]
# pass; DVE also carries the 5 logit passes, so ACT takes 13 squares.
S_ENG = ["act", "dve", "act", "dve", "act"]                     # 5 s slots
Q_ENG = ["act", "dve", "act", "act", "dve", "act", "dve", "act",
         "act", "dve", "act", "dve", "act", "act", "act"]       # 15 q slots
# last episode: its chunk-b work is the exposed tail, so balance it evenly
Q_ENG_LAST = Q_ENG[:QA] + ["act", "dve", "act", "dve", "act", "dve", "dve"]


def _build_body(ctx: ExitStack, tc: "tile.TileContext", i1, i2, wbc, msk, out):
    nc = tc.nc

    const_pool = ctx.enter_context(tc.tile_pool(name="const", bufs=1))
    qpool = ctx.enter_context(tc.tile_pool(name="qdata", bufs=1))
    spool = ctx.enter_context(tc.tile_pool(name="sdata", bufs=1))
    scr_pool = ctx.enter_context(tc.tile_pool(name="scratch", bufs=2))
    stats = ctx.enter_context(tc.tile_pool(name="stats", bufs=2))
    sel_pool = ctx.enter_context(tc.tile_pool(name="sel", bufs=3))
    sb_pool = ctx.enter_context(tc.tile_pool(name="sbwork", bufs=2))

    qm_ps = ctx.enter_context(tc.tile_pool(name="qmps", bufs=1, space="PSUM"))
    sup_ps = ctx.enter_context(tc.tile_pool(name="supps", bufs=1, space="PSUM"))
    small_ps = ctx.enter_context(tc.tile_pool(name="smallps", bufs=3, space="PSUM"))

    # ---- constants (host-precomputed, landed via SP HWDGE at t=0) ----
    masks = const_pool.tile([P, MCOLS], BF16, name="masks")
    nc.sync.dma_start(masks[:], msk)
    wb = const_pool.tile([P, C], BF16, name="wb")
    nc.sync.dma_start(wb[:], wbc)

    def qmask(j):
        return masks[:, MQ0 + j * QN: MQ0 + (j + 1) * QN]

    def smask(j):
        return masks[:, MS0 + j * NMAP: MS0 + (j + 1) * NMAP]

    ident = masks[:, MI0:MI0 + QN]      # [125, 75]; top 75 rows = I
    shotm = masks[:, MF0:MF0 + WAY]     # [125, 5]; top 25 rows live

    I32 = mybir.dt.int32

    def rsqrt1(out, x, n, tag, final_scale=None):
        """out = [final_scale *] 1/sqrt(x): bit-trick seed + 1 Newton step
        (rel err <= 1.8e-3 — small vs the bf16 data rounding, and random
        across descriptors so it averages out in the means)."""
        y = stats.tile([P, n], F32, name=f"nw_y_{tag}", tag=f"nwy_{tag}")
        t = stats.tile([P, n], F32, name=f"nw_t_{tag}", tag=f"nwt_{tag}")
        nc.vector.tensor_scalar(y.bitcast(I32)[:], x.bitcast(I32), 1, None,
                                op0=OP.arith_shift_right)
        nc.vector.tensor_scalar(y.bitcast(I32)[:], y.bitcast(I32)[:], -1,
                                0x5F3759DF, op0=OP.mult, op1=OP.add)
        nc.vector.tensor_mul(t[:], y[:], y[:])
        nc.vector.tensor_mul(t[:], t[:], x)
        nc.vector.tensor_scalar(t[:], t[:], -0.5, 1.5, op0=OP.mult, op1=OP.add)
        if final_scale is None:
            nc.vector.tensor_mul(out, y[:], t[:])
        else:
            nc.vector.scalar_tensor_tensor(out=out, in0=y[:],
                                           scalar=final_scale, in1=t[:],
                                           op0=OP.mult, op1=OP.mult)

    # ---- all-resident data tiles; one FIFO gpsimd ring orders arrivals ----
    stiles = [spool.tile([P, ST * C], BF16, name=f"s_{e}", tag=f"s{e}")
              for e in range(E)]
    qatiles = [qpool.tile([P, QA * C], BF16, name=f"qa_{e}", tag=f"qa{e}")
               for e in range(E)]
    qbtiles = [qpool.tile([P, QB * C], BF16, name=f"qb_{e}", tag=f"qb{e}")
               for e in range(E)]

    def issue_loads(e, what):
        # SP HWDGE: one FIFO ring, no compute engine burns issue time, and
        # transfers start ~6us earlier than Pool SWDGE (no Q7 library wait)
        if what == "s":
            nc.sync.dma_start(stiles[e][:], i2[e])
        elif what == "qa":
            nc.sync.dma_start(qatiles[e][:], i1[e, :, 0:QA * C])
        else:
            nc.sync.dma_start(qbtiles[e][:], i1[e, :, QA * C:QT * C])

    def qslot(e, j):
        if j < QA:
            return qatiles[e][:, j * C:(j + 1) * C]
        return qbtiles[e][:, (j - QA) * C:(j - QA + 1) * C]

    def sslot(e, j):
        return stiles[e][:, j * C:(j + 1) * C]

    # Issue ALL loads up-front on the single SP ring (FIFO): arrivals are
    # strictly ordered, so the order below IS the pipeline schedule. s2/s3
    # are pulled ahead of the last q chunks so the final episode's support
    # chain completes before its query data lands (shrinks the tail).
    order = [(0, "s"), (0, "qa"), (0, "qb"), (1, "s"), (1, "qa"), (1, "qb"),
             (2, "s"), (2, "qa"), (3, "s"), (2, "qb"), (3, "qa"), (3, "qb")]
    for e, what in order:
        issue_loads(e, what)

    def norm_pass(src, acc_col, engine, tag):
        """acc_col[p] = sum_c src[p, c]^2 via the given engine. The scratch
        `out` is never read, so it's fp8 to minimize SBUF write traffic."""
        if engine == "act":
            scr = scr_pool.tile([P, C], F8, name="sq_a", tag="sq_a")
            nc.scalar.activation(scr[:], src, AF.Square, accum_out=acc_col)
        else:
            scr = scr_pool.tile([P, C], F8, name="sq_v", tag="sq_v")
            nc.vector.scalar_tensor_tensor(
                out=scr[:], in0=src, scalar=1.0, in1=src,
                op0=OP.mult, op1=OP.mult, accum_out=acc_col)

    MT0 = 32               # mt rows land at psum partition 32 (windows
    NSUP = MT0 + NMAP      # must start at 0/32/64/96); 25..32 zero pad
    est = [dict() for _ in range(E)]   # per-episode cross-block state

    def support_block(e):
        eo = e % 2
        sn2 = stats.tile([P, ST], F32, name=f"sn2_{e}", tag=f"sn2{eo}")
        rr = stats.tile([P, ST], F32, name=f"rr_{e}", tag=f"rr{eo}")
        for j in range(ST):
            # logits pass on DVE: rr[:, j] = sum_c s*wb
            pscr = scr_pool.tile([P, C], F8, name="lg_v", tag="lg_v")
            nc.vector.scalar_tensor_tensor(
                out=pscr[:], in0=sslot(e, j), scalar=1.0, in1=wb[:],
                op0=OP.mult, op1=OP.mult, accum_out=rr[:, j:j + 1])
            norm_pass(sslot(e, j), sn2[:, j:j + 1], S_ENG[j], f"s{j}")

        # sinv = 1/sqrt(sn2) (raw, no 1/25 factor)
        sinv = stats.tile([P, ST], F32, name=f"sinv_{e}", tag=f"sinv{eo}")
        rsqrt1(sinv[:], sn2[:], ST, f"s{eo}")

        # softmax over hw within each map (logits tiny: no max-shift)
        lg = stats.tile([P, ST], F32, name=f"lgt_{e}", tag=f"lgt{eo}")
        nc.vector.tensor_mul(lg[:], rr[:], sinv[:])
        el = stats.tile([P, ST], BF16, name=f"el_{e}", tag=f"el{eo}")
        nc.scalar.activation(el[:], lg[:], AF.Exp)
        sums = small_ps.tile([NMAP, 1], F32, name=f"sums_{e}", tag="smallps")
        for j in range(ST):
            nc.tensor.matmul(sums[:], smask(j), el[:, j:j + 1],
                             start=(j == 0), stop=(j == ST - 1))
        rec = stats.tile([NMAP, 1], F32, name=f"rec_{e}", tag=f"rec{eo}")
        nc.vector.reciprocal(rec[:], sums[:])
        # GAMMA folds into the Prelu scale: leaky(g*x) = g*leaky(x) for g>0
        rec5 = stats.tile([NMAP, 1], F32, name=f"rec5_{e}", tag=f"rec5{eo}")
        nc.vector.tensor_scalar_mul(rec5[:], rec[:], GAMMA)
        # unnormalized att weights and hw-mean weights
        uw = stats.tile([P, ST], F32, name=f"uw_{e}", tag=f"uw{eo}")
        nc.vector.tensor_mul(uw[:], el[:], sinv[:])
        sinv04 = stats.tile([P, ST], F32, name=f"sv4_{e}", tag=f"sv4{eo}")
        nc.vector.tensor_scalar_mul(sinv04[:], sinv[:], 1.0 / HW)

        # fused att-sum + hw-mean: lhsT = [ut | pad | mt] -> psum [57, 320]
        # x2 halves. All 5 slot masks built in 2 broadcast-read TTs + one
        # strided pad memset (vs 10 TensorScalarPtr ops).
        sp = [sup_ps.tile([NSUP, CH], F32, name=f"sup{h}_{e}",
                          tag=f"sup{h}") for h in range(2)]
        umt = sel_pool.tile([P, ST * NSUP], BF16, name=f"umt_{e}", tag="umt")
        u3 = umt[:].rearrange("p (j k) -> p j k", j=ST)
        sm3 = masks[:, MS0:MS0 + ST * NMAP].rearrange("p (j k) -> p j k", j=ST)
        nc.vector.tensor_mul(u3[:, :, 0:NMAP], sm3,
                             uw[:].unsqueeze(2).to_broadcast((P, ST, NMAP)))
        nc.vector.memset(u3[:, :, NMAP:MT0], 0.0)
        nc.vector.tensor_mul(u3[:, :, MT0:NSUP], sm3,
                             sinv04[:].unsqueeze(2).to_broadcast((P, ST, NMAP)))
        for j in range(ST):
            for h in range(2):
                nc.tensor.matmul(sp[h][:], umt[:, j * NSUP:(j + 1) * NSUP],
                                 sslot(e, j)[:, CH * h:CH * (h + 1)],
                                 start=(j == 0), stop=(j == ST - 1))
        # evac: lk = 5*leaky(cg * rec) fused on ACT; fp = lk + sm on DVE
        lk = sb_pool.tile([NMAP, C], BF16, name=f"lk_{e}", tag="lk")
        for h in range(2):
            nc.scalar.activation(lk[:, CH * h:CH * (h + 1)], sp[h][0:NMAP, :],
                                 AF.Prelu, scale=rec5[:, 0:1], alpha=SLOPE)
        fp = sb_pool.tile([NMAP, C], BF16, name=f"fp_{e}", tag="fp")
        for h in range(2):
            nc.vector.tensor_add(fp[:, CH * h:CH * (h + 1)],
                                 lk[:, CH * h:CH * (h + 1)],
                                 sp[h][MT0:NSUP, :])
        # prototype per way (shot-mean) then transpose to [c, way]
        feat_sb = sb_pool.tile([WAY, C], BF16, name=f"feat_{e}", tag="feat")
        for h in range(2):
            f_ps = small_ps.tile([WAY, CH], F32, name=f"f_ps{h}", tag="smallps")
            nc.tensor.matmul(f_ps[:], shotm[0:NMAP, :], fp[:, CH * h:CH * (h + 1)])
            nc.scalar.copy(feat_sb[:, CH * h:CH * (h + 1)], f_ps[:])
        ftT = sb_pool.tile([128, WAY * 5], BF16, name=f"ftT_{e}", tag="ftT")
        for cc in range(5):
            t_ps = small_ps.tile([128, WAY], BF16, name="tf_ps", tag="smallps")
            nc.tensor.transpose(t_ps[:], feat_sb[:, 128 * cc:128 * (cc + 1)],
                                ident[0:WAY, 0:WAY])
            nc.vector.tensor_copy(ftT[:, WAY * cc:WAY * (cc + 1)], t_ps[:])
        est[e]["ftT"] = ftT

    def qslots_mm(e, lo, hi, qiv, qm, tag):
        # all (hi-lo) sel masks in ONE broadcast-read TT, then matmuls
        n = hi - lo
        sel = sel_pool.tile([P, n * QN], BF16, name=f"sel{tag}", tag=f"sel{tag}")
        s3 = sel[:].rearrange("p (j k) -> p j k", j=n)
        qm3 = masks[:, MQ0 + lo * QN:MQ0 + hi * QN].rearrange(
            "p (j k) -> p j k", j=n)
        nc.vector.tensor_mul(
            s3, qm3, qiv[:, lo:hi].unsqueeze(2).to_broadcast((P, n, QN)))
        for j in range(lo, hi):
            for h in range(2):
                nc.tensor.matmul(qm[h][:], sel[:, (j - lo) * QN:(j - lo + 1) * QN],
                                 qslot(e, j)[:, CH * h:CH * (h + 1)],
                                 start=(j == 0), stop=(j == QT - 1))

    def querya_block(e):
        eo = e % 2
        qeng = Q_ENG_LAST if e == E - 1 else Q_ENG
        qn2 = stats.tile([P, QT], F32, name=f"qn2_{e}", tag=f"qn2{eo}")
        qiv = stats.tile([P, QT], F32, name=f"qiv_{e}", tag=f"qiv{eo}")
        qm = [qm_ps.tile([QN, CH], F32, name=f"qm{h}_{e}", tag=f"qm{h}")
              for h in range(2)]
        est[e].update(qn2=qn2, qiv=qiv, qm=qm)
        for j in range(QA):
            norm_pass(qslot(e, j), qn2[:, j:j + 1], qeng[j], f"q{j}")
        rsqrt1(qiv[:, 0:QA], qn2[:, 0:QA], QA, f"qa{eo}",
               final_scale=1.0 / HW)
        qslots_mm(e, 0, QA, qiv, qm, "a")

    def queryb_block(e):
        eo = e % 2
        qeng = Q_ENG_LAST if e == E - 1 else Q_ENG
        qn2, qiv, qm = est[e]["qn2"], est[e]["qiv"], est[e]["qm"]
        ftT = est[e]["ftT"]
        for j in range(QA, QT):
            norm_pass(qslot(e, j), qn2[:, j:j + 1], qeng[j], f"q{j}")
        rsqrt1(qiv[:, QA:QT], qn2[:, QA:QT], QT - QA, f"qb{eo}",
               final_scale=1.0 / HW)
        qslots_mm(e, QA, QT, qiv, qm, "b")

        # endgame: evac each half then transpose the chunks it completes
        qm_sb = sb_pool.tile([QN, C], BF16, name=f"qm_sb_{e}", tag="qm_sb")
        qmT = sb_pool.tile([128, QN * 5], BF16, name=f"qmT_{e}", tag="qmT")

        def tchunk(cc):
            t_ps = small_ps.tile([128, QN], BF16, name="tq_ps", tag="smallps")
            nc.tensor.transpose(t_ps[:], qm_sb[:, 128 * cc:128 * (cc + 1)],
                                ident[0:QN, 0:QN])
            nc.vector.tensor_copy(qmT[:, QN * cc:QN * (cc + 1)], t_ps[:])

        nc.scalar.copy(qm_sb[:, 0:CH], qm[0][:])
        tchunk(0); tchunk(1)
        nc.scalar.copy(qm_sb[:, CH:C], qm[1][:])
        tchunk(2); tchunk(3); tchunk(4)
        sim_ps = small_ps.tile([QN, WAY], F32, name=f"sim_{e}", tag="smallps")
        for cc in range(5):
            nc.tensor.matmul(sim_ps[:], qmT[:, QN * cc:QN * (cc + 1)],
                             ftT[:, WAY * cc:WAY * (cc + 1)],
                             start=(cc == 0), stop=(cc == 4))
        sim_sb = sb_pool.tile([QN, WAY], F32, name=f"sim_sb_{e}", tag="sim_sb")
        nc.vector.tensor_copy(sim_sb[:], sim_ps[:])
        nc.sync.dma_start(out[e], sim_sb[:])

    # program order mirrors the DMA ring order exactly (in-order engines):
    # s0 q0a q0b s1 q1a q1b s2 q2a s3 q2b q3a q3b
    support_block(0); querya_block(0); queryb_block(0)
    support_block(1); querya_block(1); queryb_block(1)
    support_block(2); querya_block(2)
    support_block(3)
    queryb_block(2)
    querya_block(3); queryb_block(3)


def build_program():
    nc = bacc.Bacc("TRN2", target_bir_lowering=False, debug=False,
                   num_devices=NCORES)
    inp1 = nc.dram_tensor("input1", [E, P, QT * C], BF16, kind="ExternalInput")
    inp2 = nc.dram_tensor("input2", [E, P, ST * C], BF16, kind="ExternalInput")
    wbc = nc.dram_tensor("wbcast", [P, C], BF16, kind="ExternalInput")
    msk = nc.dram_tensor("masks", [P, MCOLS], BF16, kind="ExternalInput")
    out = nc.dram_tensor("sim", [E, QN, WAY], F32, kind="ExternalOutput")
    with tile.TileContext(nc) as tc, ExitStack() as ctx:
        _build_body(ctx, tc, inp1.ap(), inp2.ap(), wbc.ap(), msk.ap(), out.ap())
    nc.compile()
    return nc


_NC = None


def _get_nc():
    global _NC
    if _NC is None:
        _NC = build_program()
    return _NC


def _host_masks():
    m = np.zeros((P, MCOLS), dtype=np.float32)
    p = np.arange(P)[:, None]
    for j in range(QT):
        q = np.arange(QN)[None, :]
        d = QT * p + j - HW * q
        m[:, MQ0 + j * QN: MQ0 + (j + 1) * QN] = ((d >= 0) & (d < HW))
    for j in range(ST):
        mm = np.arange(NMAP)[None, :]
        d = ST * p + j - HW * mm
        m[:, MS0 + j * NMAP: MS0 + (j + 1) * NMAP] = ((d >= 0) & (d < HW))
    m[0:QN, MI0:MI0 + QN] = np.eye(QN, dtype=np.float32)
    mm = np.arange(NMAP)[:, None]
    w = np.arange(WAY)[None, :]
    m[0:NMAP, MF0:MF0 + WAY] = (mm // SHOT == w) * (1.0 / SHOT)
    return m.astype(ml_dtypes.bfloat16)


_MASKS = _host_masks()


def shard_inputs(input1, input2, rpn_w, rpn_b=None):
    """Shard over episodes; [E, 1875, 640] -> [E, 125, 15*640] is a pure
    reshape (descriptor d = 15p + j, slots consecutive in DRAM)."""
    i1 = np.asarray(input1, dtype=np.float32).reshape(B, P, QT * C)
    i1 = i1.astype(ml_dtypes.bfloat16)
    i2 = np.asarray(input2, dtype=np.float32).reshape(B, P, ST * C)
    i2 = i2.astype(ml_dtypes.bfloat16)
    w = np.asarray(rpn_w, dtype=np.float32).reshape(1, C)
    wbc = np.ascontiguousarray(
        np.broadcast_to(w, (P, C)).astype(ml_dtypes.bfloat16))
    in_maps = []
    for i in range(NCORES):
        in_maps.append({
            "input1": np.ascontiguousarray(i1[E * i:E * (i + 1)]),
            "input2": np.ascontiguousarray(i2[E * i:E * (i + 1)]),
            "wbcast": wbc,
            "masks": _MASKS,
        })
    return in_maps


def _ensure_ntff_hook():
    """Install the NTFF profile hook (the image's antenv lacks axon_hooks)."""
    import types
    import antenv

    if "antenv.axon_hooks" not in sys.modules:
        mod = types.ModuleType("antenv.axon_hooks")
        mod._hook = None
        mod.set_axon_ntff_profile_hook = lambda h: setattr(mod, "_hook", h)
        mod.get_axon_ntff_profile_hook = lambda: mod._hook
        sys.modules["antenv.axon_hooks"] = mod
        antenv.axon_hooks = mod
    mod = sys.modules["antenv.axon_hooks"]
    if mod.get_axon_ntff_profile_hook() is None:
        from trn_agent_boot.trn_boot import _ntff_profile_via_ctypes
        hook = _ntff_profile_via_ctypes("/opt/axon/libaxon_pjrt.so")
        if hook is not None:
            mod.set_axon_ntff_profile_hook(hook)


def kernel(input1, input2, rpn_w, rpn_b=None, **run_kwargs):
    if run_kwargs.get("trace"):
        _ensure_ntff_hook()
    nc = _get_nc()
    in_maps = shard_inputs(input1, input2, rpn_w)
    res = run_bass_kernel_spmd(nc, in_maps, list(range(NCORES)), **run_kwargs)
    out = np.concatenate([r["sim"] for r in res.results], axis=0)
    if run_kwargs:
        kernel.last_results = res
    return out.astype(np.float32)


# revision 28
# speedup vs baseline: 1.1830x; 1.1830x over previous
"""MetaBaseline (retrieval_knn) Trainium2 kernel — bf16 edition.

Computation (per episode b):
  q  = l2norm(input1[b])            # [75, 25, 640] over channel
  s  = l2norm(input2[b])            # [5, 5, 25, 640]
  att = softmax_hw(s @ rpn_w)       # rpn_b is softmax-invariant
  cg  = leaky(sum_hw(att * s))
  feat = mean_shot(mean_hw(s) + 5 * cg)
  sim[b] = mean_hw(q) @ feat.T      # [75, 5]

Sharding: data-parallel over episodes, 4 per core on 8 cores.

Design notes (v2):
- All bulk data bf16 (host-rounded): halves HBM traffic (the kernel is
  memory-regime), enables the DVE 2x packed mode for the square-accum
  passes, and PE streams bf16 rhs at 1 col/cycle like f32r.
- One FIFO SWDGE ring (gpsimd) carries every big load in arrival order
  s0,q0a,q0b,s1,q1a,... so episode e's support data lands before its
  query chunks and episodes complete in order at full aggregate DMA
  bandwidth. Later issues are interleaved with Pool compute to keep the
  ring backlog bounded. Everything is SBUF-resident (~120KB/partition).
- Square-accum norm passes are split across ACT (Square+accum), DVE
  (scalar_tensor_tensor 2x), and Pool; support logits run on Pool.
- rsqrt = Exp(-0.5*Ln(x)) on ACT (2 ops per batch; table set
  natural_log_exp_and_others holds Ln/Exp/Square/Copy/Prelu). The 1/25
  hw-mean factor rides in the Exp bias for the query side.
- Support att-sum and hw-mean fuse into ONE matmul per slot-half with a
  concatenated [ut|mt] mask -> [50, 320] psum; evacuation fuses
  psum-read * softmax-recip * leaky into a single ACT Prelu per half.
- Staircase masks / identity / shot-mean matrices are host-precomputed
  and DMA'd (tiny) instead of burning gpsimd affine_selects.
"""

import sys
from contextlib import ExitStack

sys.path.insert(0, "/opt/trn_rl_repo")

import ml_dtypes
import numpy as np

import concourse.bass as bass
import concourse.tile as tile
from concourse import bacc, mybir
from concourse.bass_utils import run_bass_kernel_spmd

F32 = mybir.dt.float32
BF16 = mybir.dt.bfloat16
F8 = mybir.dt.float8e4      # scratch-only: halves useless SBUF writes
OP = mybir.AluOpType
AF = mybir.ActivationFunctionType

# Problem constants (fixed by the problem statement).
B, QN, WAY, SHOT, HH, WW, C = 32, 75, 5, 5, 5, 5, 640
NCORES = 8
E = B // NCORES        # 4 episodes per core
HW = HH * WW           # 25 spatial positions
P = 125                # descriptors per tile (5 groups of 25)
QT = 15                # query slots / episode (1875 = 15*125)
ST = 5                 # support slots / episode (625 = 5*125)
NMAP = WAY * SHOT      # 25 support maps / episode
GAMMA = 5.0
SLOPE = 0.01
CH = C // 2            # 320-column psum-bank-sized chunk
QA = 8                 # query slots in DMA chunk a (slots 0..7)
QB = QT - QA           # chunk b (slots 8..14)

# masks tensor column layout: 15 qmasks | 5 smasks | ident75 | shotm
MQ0 = 0                       # qmask_j at MQ0 + j*QN, width QN
MS0 = QT * QN                 # smask_j at MS0 + j*NMAP, width NMAP
MI0 = MS0 + ST * NMAP         # identity 75x75
MF0 = MI0 + QN                # shotm [25, 5]
MCOLS = MF0 + WAY

# norm-pass engine schedule (which engine squares+accumulates each slot).
# Pool cannot reduce over the free axis, so only ACT/DVE carry these.
# Measured: ACT Square+acc-read ~0.91us, DVE STT ~0.74us per [125,640]
# pass; DVE also carries the 5 logit passes, so ACT takes 13 squares.
S_ENG = ["act", "dve", "act", "dve", "act"]                     # 5 s slots
Q_ENG = ["act", "dve", "act", "act", "dve", "act", "dve", "act",
         "act", "dve", "act", "dve", "act", "act", "act"]       # 15 q slots
# last episode: its chunk-b work is the exposed tail, so balance it evenly
Q_ENG_LAST = Q_ENG[:QA] + ["act", "dve", "act", "dve", "act", "dve", "dve"]


def _build_body(ctx: ExitStack, tc: "tile.TileContext", i1, i2, wbc, msk, out):
    nc = tc.nc

    const_pool = ctx.enter_context(tc.tile_pool(name="const", bufs=1))
    qpool = ctx.enter_context(tc.tile_pool(name="qdata", bufs=1))
    spool = ctx.enter_context(tc.tile_pool(name="sdata", bufs=1))
    scr_pool = ctx.enter_context(tc.tile_pool(name="scratch", bufs=2))
    stats = ctx.enter_context(tc.tile_pool(name="stats", bufs=2))
    sel_pool = ctx.enter_context(tc.tile_pool(name="sel", bufs=3))
    sb_pool = ctx.enter_context(tc.tile_pool(name="sbwork", bufs=2))

    qm_ps = ctx.enter_context(tc.tile_pool(name="qmps", bufs=1, space="PSUM"))
    sup_ps = ctx.enter_context(tc.tile_pool(name="supps", bufs=1, space="PSUM"))
    small_ps = ctx.enter_context(tc.tile_pool(name="smallps", bufs=3, space="PSUM"))

    # ---- constants (host-precomputed, landed via SP HWDGE at t=0) ----
    masks = const_pool.tile([P, MCOLS], BF16, name="masks")
    nc.sync.dma_start(masks[:], msk)
    wb = const_pool.tile([P, C], BF16, name="wb")
    nc.sync.dma_start(wb[:], wbc)

    def qmask(j):
        return masks[:, MQ0 + j * QN: MQ0 + (j + 1) * QN]

    def smask(j):
        return masks[:, MS0 + j * NMAP: MS0 + (j + 1) * NMAP]

    ident = masks[:, MI0:MI0 + QN]      # [125, 75]; top 75 rows = I
    shotm = masks[:, MF0:MF0 + WAY]     # [125, 5]; top 25 rows live

    I32 = mybir.dt.int32

    def rsqrt1(out, x, n, tag, final_scale=None):
        """out = [final_scale *] 1/sqrt(x): bit-trick seed + 1 Newton step
        (rel err <= 1.8e-3 — small vs the bf16 data rounding, and random
        across descriptors so it averages out in the means)."""
        y = stats.tile([P, n], F32, name=f"nw_y_{tag}", tag=f"nwy_{tag}")
        t = stats.tile([P, n], F32, name=f"nw_t_{tag}", tag=f"nwt_{tag}")
        nc.vector.tensor_scalar(y.bitcast(I32)[:], x.bitcast(I32), 1, None,
                                op0=OP.arith_shift_right)
        nc.vector.tensor_scalar(y.bitcast(I32)[:], y.bitcast(I32)[:], -1,
                                0x5F3759DF, op0=OP.mult, op1=OP.add)
        nc.vector.tensor_mul(t[:], y[:], y[:])
        nc.vector.tensor_mul(t[:], t[:], x)
        nc.vector.tensor_scalar(t[:], t[:], -0.5, 1.5, op0=OP.mult, op1=OP.add)
        if final_scale is None:
            nc.vector.tensor_mul(out, y[:], t[:])
        else:
            nc.vector.scalar_tensor_tensor(out=out, in0=y[:],
                                           scalar=final_scale, in1=t[:],
                                           op0=OP.mult, op1=OP.mult)

    # ---- all-resident data tiles; one FIFO gpsimd ring orders arrivals ----
    stiles = [spool.tile([P, ST * C], BF16, name=f"s_{e}", tag=f"s{e}")
              for e in range(E)]
    qatiles = [qpool.tile([P, QA * C], BF16, name=f"qa_{e}", tag=f"qa{e}")
               for e in range(E)]
    qbtiles = [qpool.tile([P, QB * C], BF16, name=f"qb_{e}", tag=f"qb{e}")
               for e in range(E)]

    def issue_loads(e, what):
        # Pool SWDGE ring: measured ~160-200 GB/s sustained vs ~130 for the
        # SP HWDGE ring. Pool has no other work, so the issue cost is free.
        if what == "s":
            nc.gpsimd.dma_start(stiles[e][:], i2[e])
        elif what == "qa":
            nc.gpsimd.dma_start(qatiles[e][:], i1[e, :, 0:QA * C])
        else:
            nc.gpsimd.dma_start(qbtiles[e][:], i1[e, :, QA * C:QT * C])

    def qslot(e, j):
        if j < QA:
            return qatiles[e][:, j * C:(j + 1) * C]
        return qbtiles[e][:, (j - QA) * C:(j - QA + 1) * C]

    def sslot(e, j):
        return stiles[e][:, j * C:(j + 1) * C]

    # Issue ALL loads up-front on the single SP ring (FIFO): arrivals are
    # strictly ordered, so the order below IS the pipeline schedule. s2/s3
    # are pulled ahead of the last q chunks so the final episode's support
    # chain completes before its query data lands (shrinks the tail).
    order = [(0, "s"), (0, "qa"), (0, "qb"), (1, "s"), (1, "qa"), (1, "qb"),
             (2, "s"), (2, "qa"), (3, "s"), (2, "qb"), (3, "qa"), (3, "qb")]
    for e, what in order:
        issue_loads(e, what)

    def norm_pass(src, acc_col, engine, tag):
        """acc_col[p] = sum_c src[p, c]^2 via the given engine. The scratch
        `out` is never read, so it's fp8 to minimize SBUF write traffic."""
        if engine == "act":
            scr = scr_pool.tile([P, C], F8, name="sq_a", tag="sq_a")
            nc.scalar.activation(scr[:], src, AF.Square, accum_out=acc_col)
        else:
            scr = scr_pool.tile([P, C], F8, name="sq_v", tag="sq_v")
            nc.vector.scalar_tensor_tensor(
                out=scr[:], in0=src, scalar=1.0, in1=src,
                op0=OP.mult, op1=OP.mult, accum_out=acc_col)

    MT0 = 32               # mt rows land at psum partition 32 (windows
    NSUP = MT0 + NMAP      # must start at 0/32/64/96); 25..32 zero pad
    est = [dict() for _ in range(E)]   # per-episode cross-block state

    def support_block(e):
        eo = e % 2
        sn2 = stats.tile([P, ST], F32, name=f"sn2_{e}", tag=f"sn2{eo}")
        rr = stats.tile([P, ST], F32, name=f"rr_{e}", tag=f"rr{eo}")
        for j in range(ST):
            # logits pass on DVE: rr[:, j] = sum_c s*wb
            pscr = scr_pool.tile([P, C], F8, name="lg_v", tag="lg_v")
            nc.vector.scalar_tensor_tensor(
                out=pscr[:], in0=sslot(e, j), scalar=1.0, in1=wb[:],
                op0=OP.mult, op1=OP.mult, accum_out=rr[:, j:j + 1])
            norm_pass(sslot(e, j), sn2[:, j:j + 1], S_ENG[j], f"s{j}")

        # sinv = 1/sqrt(sn2) (raw, no 1/25 factor)
        sinv = stats.tile([P, ST], F32, name=f"sinv_{e}", tag=f"sinv{eo}")
        rsqrt1(sinv[:], sn2[:], ST, f"s{eo}")

        # softmax over hw within each map (logits tiny: no max-shift)
        lg = stats.tile([P, ST], F32, name=f"lgt_{e}", tag=f"lgt{eo}")
        nc.vector.tensor_mul(lg[:], rr[:], sinv[:])
        el = stats.tile([P, ST], BF16, name=f"el_{e}", tag=f"el{eo}")
        nc.scalar.activation(el[:], lg[:], AF.Exp)
        sums = small_ps.tile([NMAP, 1], F32, name=f"sums_{e}", tag="smallps")
        for j in range(ST):
            nc.tensor.matmul(sums[:], smask(j), el[:, j:j + 1],
                             start=(j == 0), stop=(j == ST - 1))
        rec = stats.tile([NMAP, 1], F32, name=f"rec_{e}", tag=f"rec{eo}")
        nc.vector.reciprocal(rec[:], sums[:])
        # GAMMA folds into the Prelu scale: leaky(g*x) = g*leaky(x) for g>0
        rec5 = stats.tile([NMAP, 1], F32, name=f"rec5_{e}", tag=f"rec5{eo}")
        nc.vector.tensor_scalar_mul(rec5[:], rec[:], GAMMA)
        # unnormalized att weights and hw-mean weights
        uw = stats.tile([P, ST], F32, name=f"uw_{e}", tag=f"uw{eo}")
        nc.vector.tensor_mul(uw[:], el[:], sinv[:])
        sinv04 = stats.tile([P, ST], F32, name=f"sv4_{e}", tag=f"sv4{eo}")
        nc.vector.tensor_scalar_mul(sinv04[:], sinv[:], 1.0 / HW)

        # fused att-sum + hw-mean: lhsT = [ut | pad | mt] -> psum [57, 320]
        # x2 halves. All 5 slot masks built in 2 broadcast-read TTs + one
        # strided pad memset (vs 10 TensorScalarPtr ops).
        sp = [sup_ps.tile([NSUP, CH], F32, name=f"sup{h}_{e}",
                          tag=f"sup{h}") for h in range(2)]
        umt = sel_pool.tile([P, ST * NSUP], BF16, name=f"umt_{e}", tag="umt")
        u3 = umt[:].rearrange("p (j k) -> p j k", j=ST)
        sm3 = masks[:, MS0:MS0 + ST * NMAP].rearrange("p (j k) -> p j k", j=ST)
        nc.vector.tensor_mul(u3[:, :, 0:NMAP], sm3,
                             uw[:].unsqueeze(2).to_broadcast((P, ST, NMAP)))
        nc.vector.memset(u3[:, :, NMAP:MT0], 0.0)
        nc.vector.tensor_mul(u3[:, :, MT0:NSUP], sm3,
                             sinv04[:].unsqueeze(2).to_broadcast((P, ST, NMAP)))
        for j in range(ST):
            for h in range(2):
                nc.tensor.matmul(sp[h][:], umt[:, j * NSUP:(j + 1) * NSUP],
                                 sslot(e, j)[:, CH * h:CH * (h + 1)],
                                 start=(j == 0), stop=(j == ST - 1))
        # evac: lk = 5*leaky(cg * rec) fused on ACT; fp = lk + sm on DVE
        lk = sb_pool.tile([NMAP, C], BF16, name=f"lk_{e}", tag="lk")
        for h in range(2):
            nc.scalar.activation(lk[:, CH * h:CH * (h + 1)], sp[h][0:NMAP, :],
                                 AF.Prelu, scale=rec5[:, 0:1], alpha=SLOPE)
        fp = sb_pool.tile([NMAP, C], BF16, name=f"fp_{e}", tag="fp")
        for h in range(2):
            nc.vector.tensor_add(fp[:, CH * h:CH * (h + 1)],
                                 lk[:, CH * h:CH * (h + 1)],
                                 sp[h][MT0:NSUP, :])
        # prototype per way (shot-mean) then transpose to [c, way]
        feat_sb = sb_pool.tile([WAY, C], BF16, name=f"feat_{e}", tag="feat")
        for h in range(2):
            f_ps = small_ps.tile([WAY, CH], F32, name=f"f_ps{h}", tag="smallps")
            nc.tensor.matmul(f_ps[:], shotm[0:NMAP, :], fp[:, CH * h:CH * (h + 1)])
            nc.scalar.copy(feat_sb[:, CH * h:CH * (h + 1)], f_ps[:])
        ftT = sb_pool.tile([128, WAY * 5], BF16, name=f"ftT_{e}", tag="ftT")
        for cc in range(5):
            t_ps = small_ps.tile([128, WAY], BF16, name="tf_ps", tag="smallps")
            nc.tensor.transpose(t_ps[:], feat_sb[:, 128 * cc:128 * (cc + 1)],
                                ident[0:WAY, 0:WAY])
            nc.vector.tensor_copy(ftT[:, WAY * cc:WAY * (cc + 1)], t_ps[:])
        est[e]["ftT"] = ftT

    def qslots_mm(e, lo, hi, qiv, qm, tag):
        # all (hi-lo) sel masks in ONE broadcast-read TT, then matmuls
        n = hi - lo
        sel = sel_pool.tile([P, n * QN], BF16, name=f"sel{tag}", tag=f"sel{tag}")
        s3 = sel[:].rearrange("p (j k) -> p j k", j=n)
        qm3 = masks[:, MQ0 + lo * QN:MQ0 + hi * QN].rearrange(
            "p (j k) -> p j k", j=n)
        nc.vector.tensor_mul(
            s3, qm3, qiv[:, lo:hi].unsqueeze(2).to_broadcast((P, n, QN)))
        for j in range(lo, hi):
            for h in range(2):
                nc.tensor.matmul(qm[h][:], sel[:, (j - lo) * QN:(j - lo + 1) * QN],
                                 qslot(e, j)[:, CH * h:CH * (h + 1)],
                                 start=(j == 0), stop=(j == QT - 1))

    def querya_block(e):
        eo = e % 2
        qeng = Q_ENG_LAST if e == E - 1 else Q_ENG
        qn2 = stats.tile([P, QT], F32, name=f"qn2_{e}", tag=f"qn2{eo}")
        qiv = stats.tile([P, QT], F32, name=f"qiv_{e}", tag=f"qiv{eo}")
        qm = [qm_ps.tile([QN, CH], F32, name=f"qm{h}_{e}", tag=f"qm{h}")
              for h in range(2)]
        est[e].update(qn2=qn2, qiv=qiv, qm=qm)
        for j in range(QA):
            norm_pass(qslot(e, j), qn2[:, j:j + 1], qeng[j], f"q{j}")
        rsqrt1(qiv[:, 0:QA], qn2[:, 0:QA], QA, f"qa{eo}",
               final_scale=1.0 / HW)
        qslots_mm(e, 0, QA, qiv, qm, "a")

    def queryb_block(e):
        eo = e % 2
        qeng = Q_ENG_LAST if e == E - 1 else Q_ENG
        qn2, qiv, qm = est[e]["qn2"], est[e]["qiv"], est[e]["qm"]
        ftT = est[e]["ftT"]
        for j in range(QA, QT):
            norm_pass(qslot(e, j), qn2[:, j:j + 1], qeng[j], f"q{j}")
        rsqrt1(qiv[:, QA:QT], qn2[:, QA:QT], QT - QA, f"qb{eo}",
               final_scale=1.0 / HW)
        qslots_mm(e, QA, QT, qiv, qm, "b")

        # endgame: evac each half then transpose the chunks it completes
        qm_sb = sb_pool.tile([QN, C], BF16, name=f"qm_sb_{e}", tag="qm_sb")
        qmT = sb_pool.tile([128, QN * 5], BF16, name=f"qmT_{e}", tag="qmT")

        def tchunk(cc):
            t_ps = small_ps.tile([128, QN], BF16, name="tq_ps", tag="smallps")
            nc.tensor.transpose(t_ps[:], qm_sb[:, 128 * cc:128 * (cc + 1)],
                                ident[0:QN, 0:QN])
            nc.vector.tensor_copy(qmT[:, QN * cc:QN * (cc + 1)], t_ps[:])

        nc.scalar.copy(qm_sb[:, 0:CH], qm[0][:])
        tchunk(0); tchunk(1)
        nc.scalar.copy(qm_sb[:, CH:C], qm[1][:])
        tchunk(2); tchunk(3); tchunk(4)
        sim_ps = small_ps.tile([QN, WAY], F32, name=f"sim_{e}", tag="smallps")
        for cc in range(5):
            nc.tensor.matmul(sim_ps[:], qmT[:, QN * cc:QN * (cc + 1)],
                             ftT[:, WAY * cc:WAY * (cc + 1)],
                             start=(cc == 0), stop=(cc == 4))
        sim_sb = sb_pool.tile([QN, WAY], F32, name=f"sim_sb_{e}", tag="sim_sb")
        nc.vector.tensor_copy(sim_sb[:], sim_ps[:])
        nc.sync.dma_start(out[e], sim_sb[:])

    # program order mirrors the DMA ring order exactly (in-order engines):
    # s0 q0a q0b s1 q1a q1b s2 q2a s3 q2b q3a q3b
    support_block(0); querya_block(0); queryb_block(0)
    support_block(1); querya_block(1); queryb_block(1)
    support_block(2); querya_block(2)
    support_block(3)
    queryb_block(2)
    querya_block(3); queryb_block(3)


def build_program():
    nc = bacc.Bacc("TRN2", target_bir_lowering=False, debug=False,
                   num_devices=NCORES)
    inp1 = nc.dram_tensor("input1", [E, P, QT * C], BF16, kind="ExternalInput")
    inp2 = nc.dram_tensor("input2", [E, P, ST * C], BF16, kind="ExternalInput")
    wbc = nc.dram_tensor("wbcast", [P, C], BF16, kind="ExternalInput")
    msk = nc.dram_tensor("masks", [P, MCOLS], BF16, kind="ExternalInput")
    out = nc.dram_tensor("sim", [E, QN, WAY], F32, kind="ExternalOutput")
    with tile.TileContext(nc) as tc, ExitStack() as ctx:
        _build_body(ctx, tc, inp1.ap(), inp2.ap(), wbc.ap(), msk.ap(), out.ap())
    nc.compile()
    return nc


_NC = None


def _get_nc():
    global _NC
    if _NC is None:
        _NC = build_program()
    return _NC


def _host_masks():
    m = np.zeros((P, MCOLS), dtype=np.float32)
    p = np.arange(P)[:, None]
    for j in range(QT):
        q = np.arange(QN)[None, :]
        d = QT * p + j - HW * q
        m[:, MQ0 + j * QN: MQ0 + (j + 1) * QN] = ((d >= 0) & (d < HW))
    for j in range(ST):
        mm = np.arange(NMAP)[None, :]
        d = ST * p + j - HW * mm
        m[:, MS0 + j * NMAP: MS0 + (j + 1) * NMAP] = ((d >= 0) & (d < HW))
    m[0:QN, MI0:MI0 + QN] = np.eye(QN, dtype=np.float32)
    mm = np.arange(NMAP)[:, None]
    w = np.arange(WAY)[None, :]
    m[0:NMAP, MF0:MF0 + WAY] = (mm // SHOT == w) * (1.0 / SHOT)
    return m.astype(ml_dtypes.bfloat16)


_MASKS = _host_masks()


def shard_inputs(input1, input2, rpn_w, rpn_b=None):
    """Shard over episodes; [E, 1875, 640] -> [E, 125, 15*640] is a pure
    reshape (descriptor d = 15p + j, slots consecutive in DRAM)."""
    i1 = np.asarray(input1, dtype=np.float32).reshape(B, P, QT * C)
    i1 = i1.astype(ml_dtypes.bfloat16)
    i2 = np.asarray(input2, dtype=np.float32).reshape(B, P, ST * C)
    i2 = i2.astype(ml_dtypes.bfloat16)
    w = np.asarray(rpn_w, dtype=np.float32).reshape(1, C)
    wbc = np.ascontiguousarray(
        np.broadcast_to(w, (P, C)).astype(ml_dtypes.bfloat16))
    in_maps = []
    for i in range(NCORES):
        in_maps.append({
            "input1": np.ascontiguousarray(i1[E * i:E * (i + 1)]),
            "input2": np.ascontiguousarray(i2[E * i:E * (i + 1)]),
            "wbcast": wbc,
            "masks": _MASKS,
        })
    return in_maps


def _ensure_ntff_hook():
    """Install the NTFF profile hook (the image's antenv lacks axon_hooks)."""
    import types
    import antenv

    if "antenv.axon_hooks" not in sys.modules:
        mod = types.ModuleType("antenv.axon_hooks")
        mod._hook = None
        mod.set_axon_ntff_profile_hook = lambda h: setattr(mod, "_hook", h)
        mod.get_axon_ntff_profile_hook = lambda: mod._hook
        sys.modules["antenv.axon_hooks"] = mod
        antenv.axon_hooks = mod
    mod = sys.modules["antenv.axon_hooks"]
    if mod.get_axon_ntff_profile_hook() is None:
        from trn_agent_boot.trn_boot import _ntff_profile_via_ctypes
        hook = _ntff_profile_via_ctypes("/opt/axon/libaxon_pjrt.so")
        if hook is not None:
            mod.set_axon_ntff_profile_hook(hook)


def kernel(input1, input2, rpn_w, rpn_b=None, **run_kwargs):
    if run_kwargs.get("trace"):
        _ensure_ntff_hook()
    nc = _get_nc()
    in_maps = shard_inputs(input1, input2, rpn_w)
    res = run_bass_kernel_spmd(nc, in_maps, list(range(NCORES)), **run_kwargs)
    out = np.concatenate([r["sim"] for r in res.results], axis=0)
    if run_kwargs:
        kernel.last_results = res
    return out.astype(np.float32)
